# revision 13
# baseline (speedup 1.0000x reference)
"""Trainium2 Bass kernel for a VGAE-style 2-layer GCN encoder (fused KL).

Math refactor: GCNConv(x, W) = A_norm @ (x @ W) + b = (A_norm @ x) @ W + b, so
only two sparse propagations are needed:
  pass A: A1x = A1 @ x (edge_score weighting), Q2 = A2 @ x (ones weighting)
          -- both from ONE gather of x[src] per edge;
          enc_t = relu(A1x @ W_enc + b_enc)
  allgather enc_t across the 8 cores
  pass B: P2 = A2 @ enc_t; then small dense heads + fused KL.

Distribution: destination-sharded, 49 blocks x 128 nodes per core (padded
50176 nodes globally, rank-major new-id order).  Sparse propagation is
edge-centric: dma_gather of source rows (4 SWDGE queues round-robin,
single_packet=False) + per-128-edge-tile scaled one-hot built in one DVE
tensor_scalar (is_equal, mult) + TensorE matmul accumulated in PSUM.  Pass A
streams both weightings as one 256-wide rhs.  Self-loops use a plain DMA of
the block's own rows instead of gather slots.
"""
import math
import os

import numpy as np

N = int(os.environ.get("GCN_N", 50000))
E = int(os.environ.get("GCN_E", 800000))
D = 128
P = 128
NCORES = 8
NBLK = int(os.environ.get("GCN_NBLK", 49))   # blocks per core
NPC = NBLK * P             # nodes per core
NPAD = NCORES * NPC        # padded node count
SPLIT = (NCORES // 2) * NPC  # gather-table A/B split row (int16 range)
EPS = 1e-9

PROP = os.environ.get("GCN_PROP_DT", "bf16")
GROUP = 4 if PROP == "bf16" else 2   # blocks per dma_gather call
NQ = 4                               # SWDGE queues


def _prop_np():
    if PROP == "bf16":
        import ml_dtypes

        return np.dtype(ml_dtypes.bfloat16)
    return np.dtype(np.float32)


# ----------------------------------------------------------------------------
# host-side graph preprocessing
# ----------------------------------------------------------------------------

def _preprocess(edge_index, edge_score):
    src = np.asarray(edge_index[0], np.int64)
    dst = np.asarray(edge_index[1], np.int64)
    w1 = np.asarray(edge_score, np.float32)

    # degrees include the self loop (weight 1 in both weightings)
    loop = np.arange(N, dtype=np.int64)
    s_all = np.concatenate([src, loop])
    d_all = np.concatenate([dst, loop])
    w_all = np.concatenate([w1, np.ones(N, np.float32)])
    deg1 = np.bincount(d_all, weights=w_all.astype(np.float64), minlength=N)
    deg2 = np.bincount(d_all, minlength=N).astype(np.float64)
    dinv1 = (1.0 / np.sqrt(deg1)).astype(np.float32)
    dinv2 = (1.0 / np.sqrt(deg2)).astype(np.float32)

    # per-REAL-edge norms; self loops handled by the per-block self tile
    norm1 = dinv1[src] * w1 * dinv1[dst]
    norm2 = dinv2[src] * dinv2[dst]
    selfn1 = dinv1 * dinv1          # weight-1 self loop under both weightings
    selfn2 = dinv2 * dinv2

    indeg = np.bincount(d_all, minlength=N).astype(np.int64)

    # -- phase 1: snake-assign nodes (padded to NPAD) to cores by in-degree --
    deg_pad = np.zeros(NPAD, np.int64)
    deg_pad[:N] = indeg
    order = np.argsort(-deg_pad, kind="stable")
    snake = np.concatenate([np.arange(NCORES), np.arange(NCORES)[::-1]])
    core_of = np.empty(NPAD, np.int32)
    core_of[order] = snake[np.arange(NPAD) % (2 * NCORES)]

    # -- phase 2: per core, pack nodes into NBLK blocks balancing (dA, dB) --
    srcA = core_of[src] < (NCORES // 2)
    dA = np.bincount(dst[srcA], minlength=N)
    dB = np.bincount(dst[~srcA], minlength=N)
    dA_pad = np.zeros(NPAD, np.int64)
    dB_pad = np.zeros(NPAD, np.int64)
    dA_pad[:N] = dA
    dB_pad[:N] = dB

    block_of = np.empty(NPAD, np.int32)
    slot_of = np.empty(NPAD, np.int32)
    for c in range(NCORES):
        ids = np.nonzero(core_of == c)[0]
        assert len(ids) == NPC, len(ids)
        ids = ids[np.argsort(-(dA_pad[ids] + dB_pad[ids]), kind="stable")]
        loadA = np.zeros(NBLK, np.int64)
        loadB = np.zeros(NBLK, np.int64)
        count = np.zeros(NBLK, np.int32)
        for nid in ids:
            a, b = dA_pad[nid], dB_pad[nid]
            cand = np.nonzero(count < P)[0]
            j = cand[np.argmin(np.maximum(loadA[cand] + a, loadB[cand] + b))]
            block_of[nid] = j
            slot_of[nid] = count[j]
            count[j] += 1
            loadA[j] += a
            loadB[j] += b
    new_row = core_of.astype(np.int64) * NPC + block_of * P + slot_of

    capA = int(math.ceil(max(
        np.bincount((core_of[dst] * NBLK + block_of[dst])[srcA],
                    minlength=NCORES * NBLK).max(), 1) / P))
    capB = int(math.ceil(max(
        np.bincount((core_of[dst] * NBLK + block_of[dst])[~srcA],
                    minlength=NCORES * NBLK).max(), 1) / P))

    # -- real-edge arrays sorted by (core, block, group) --
    key = (core_of[dst].astype(np.int64) * NBLK + block_of[dst]) * 2 + (
        ~srcA).astype(np.int64)
    eorder = np.argsort(key, kind="stable")
    key_s = key[eorder]
    return dict(
        new_row=new_row, core_of=core_of, block_of=block_of, slot_of=slot_of,
        capA=capA, capB=capB,
        srow=new_row[src][eorder], dloc=slot_of[dst][eorder],
        n1=norm1[eorder], n2=norm2[eorder],
        seg_starts=np.searchsorted(key_s, np.arange(NCORES * NBLK * 2)),
        seg_ends=np.searchsorted(key_s, np.arange(NCORES * NBLK * 2) + 1),
        selfn1=selfn1, selfn2=selfn2,
    )


def _wrap_idxs(a):
    """[num] -> [128, num/16] int16 wrapped + replicated-per-Q7-pair layout."""
    return np.tile(np.asarray(a, np.int16).reshape(-1, 16).T, (8, 1))


def _build_core_arrays(pp, prop_np):
    """Per-core idx/dstloc/norm arrays; capT = capA + capB + 1 (self tile)."""
    capA, capB = pp["capA"], pp["capB"]
    capT = capA + capB + 1
    # map (core,block,slot) -> old node id (or -1 for dummy slots)
    old_of = np.full(NPAD, -1, np.int64)
    old_of[pp["new_row"][:N] if N < NPAD else pp["new_row"]] = np.arange(
        min(N, NPAD))
    out = []
    for c in range(NCORES):
        idxA = np.zeros((NBLK, capA * P), np.int16)
        idxB = np.zeros((NBLK, capB * P), np.int16)
        dloc = np.zeros((P, NBLK * capT), np.float32)
        nrm1 = np.zeros((P, NBLK * capT), np.float32)
        nrm2 = np.zeros((P, NBLK * capT), np.float32)
        for b in range(NBLK):
            for grp in range(2):
                seg = (c * NBLK + b) * 2 + grp
                s0, s1 = pp["seg_starts"][seg], pp["seg_ends"][seg]
                n = s1 - s0
                cap = capA if grp == 0 else capB
                assert n <= cap * P, (c, b, grp, n, cap * P)
                rows = pp["srow"][s0:s1] - (0 if grp == 0 else SPLIT)
                (idxA if grp == 0 else idxB)[b, :n] = rows
                colbase = b * capT + (0 if grp == 0 else capA)
                dl = np.zeros(cap * P, np.float32)
                w1 = np.zeros(cap * P, np.float32)
                w2 = np.zeros(cap * P, np.float32)
                dl[:n] = pp["dloc"][s0:s1]
                w1[:n] = pp["n1"][s0:s1]
                w2[:n] = pp["n2"][s0:s1]
                dloc[:, colbase:colbase + cap] = dl.reshape(cap, P).T
                nrm1[:, colbase:colbase + cap] = w1.reshape(cap, P).T
                nrm2[:, colbase:colbase + cap] = w2.reshape(cap, P).T
            # self tile (last col of the block): lane p == slot p
            sc = b * capT + capT - 1
            dloc[:, sc] = np.arange(P, dtype=np.float32)
            olds = old_of[c * NPC + b * P + np.arange(P)]
            valid = olds >= 0
            nrm1[valid, sc] = pp["selfn1"][olds[valid]]
            nrm2[valid, sc] = pp["selfn2"][olds[valid]]
        out.append(dict(
            idxA=_wrap_idxs(idxA.reshape(-1)),
            idxB=_wrap_idxs(idxB.reshape(-1)),
            dloc=dloc, nrm1=nrm1, nrm2=nrm2,
        ))
    return out


# ----------------------------------------------------------------------------
# bass program
# ----------------------------------------------------------------------------

_PROGRAM_CACHE = {}


def _build_program(capA, capB):
    stage = os.environ.get("GCN_STAGE", "full")
    key = (capA, capB, PROP, stage)
    if key in _PROGRAM_CACHE:
        return _PROGRAM_CACHE[key]

    import concourse.bacc as bacc
    import concourse.tile as tile
    from concourse import mybir

    DT = mybir.dt.bfloat16 if PROP == "bf16" else mybir.dt.float32
    F32 = mybir.dt.float32
    I16 = mybir.dt.int16
    capT = capA + capB + 1
    AluOp = mybir.AluOpType
    Act = mybir.ActivationFunctionType

    nc = bacc.Bacc("TRN2", target_bir_lowering=False, debug=False,
                   num_devices=NCORES, num_swdge_queues=NQ)

    # inputs
    x_tab = nc.dram_tensor("x_tab", [NPAD, D], DT, kind="ExternalInput")
    x_self_d = nc.dram_tensor("x_self", [NPC, D], DT, kind="ExternalInput")
    idxA_d = nc.dram_tensor("idxA", [128, NBLK * capA * 8], I16, kind="ExternalInput")
    idxB_d = nc.dram_tensor("idxB", [128, NBLK * capB * 8], I16, kind="ExternalInput")
    dloc_d = nc.dram_tensor("dloc", [P, NBLK * capT], F32, kind="ExternalInput")
    nrm1_d = nc.dram_tensor("nrm1", [P, NBLK * capT], F32, kind="ExternalInput")
    nrm2_d = nc.dram_tensor("nrm2", [P, NBLK * capT], F32, kind="ExternalInput")
    iota_d = nc.dram_tensor("iota", [P, P], DT, kind="ExternalInput")
    ident_d = nc.dram_tensor("ident", [P, P], DT, kind="ExternalInput")
    epsT_d = nc.dram_tensor("epsT", [P, NPC], F32, kind="ExternalInput")
    w_names = ["W_enc", "W_mu", "W_std", "W_prior", "W_pm", "W_ps"]
    w_d = {n: nc.dram_tensor(n, [D, D], DT, kind="ExternalInput") for n in w_names}
    b_names = ["b_enc", "b_mu", "b_std", "b_prior", "bpm2", "bps2"]
    b_d = {n: nc.dram_tensor(n, [D, 1], F32, kind="ExternalInput") for n in b_names}

    # outputs
    confzT_d = nc.dram_tensor("confzT", [P, NPC], F32, kind="ExternalOutput")
    klcols_d = nc.dram_tensor("klcols", [P, NBLK], F32, kind="ExternalOutput")

    NGRP = (NBLK + GROUP - 1) // GROUP
    groups = [list(range(g * GROUP, min((g + 1) * GROUP, NBLK)))
              for g in range(NGRP)]
    qctr = [0]

    with tile.TileContext(nc) as tc:
        with (
            tc.tile_pool(name="const", bufs=1) as constp,
            tc.tile_pool(name="meta", bufs=1) as metap,
            tc.tile_pool(name="gath", bufs=2) as gathp,
            tc.tile_pool(name="oh", bufs=4) as ohp,
            tc.tile_pool(name="blk", bufs=3) as blkp,
            tc.tile_pool(name="ps", bufs=2, space="PSUM") as psp,
            tc.tile_pool(name="ps2", bufs=4, space="PSUM") as ps2p,
            tc.tile_pool(name="dram", bufs=1, space="DRAM") as dramp,
        ):
            # ---- resident constants / metadata ----
            iota_t = constp.tile([P, P], DT)
            nc.sync.dma_start(iota_t[:], iota_d[:])
            ident_t = constp.tile([P, P], DT)
            nc.sync.dma_start(ident_t[:], ident_d[:])
            ones_t = constp.tile([P, 1], F32)
            nc.vector.memset(ones_t[:], 1.0)
            epsc_t = constp.tile([P, 1], F32)
            nc.vector.memset(epsc_t[:], EPS)
            w_t = {}
            for n in w_names:
                w_t[n] = constp.tile([D, D], DT, name=f"w_{n}")
                nc.sync.dma_start(w_t[n][:], w_d[n][:])
            b_t = {}
            for n in b_names:
                b_t[n] = constp.tile([D, 1], F32, name=f"b_{n}")
                nc.sync.dma_start(b_t[n][:], b_d[n][:])
            idxA_t = metap.tile([128, NBLK * capA * 8], I16)
            nc.sync.dma_start(idxA_t[:], idxA_d[:])
            idxB_t = metap.tile([128, NBLK * capB * 8], I16)
            nc.sync.dma_start(idxB_t[:], idxB_d[:])
            dloc_t = metap.tile([P, NBLK * capT], F32)
            nc.sync.dma_start(dloc_t[:], dloc_d[:])
            nrm1_t = metap.tile([P, NBLK * capT], F32)
            nc.sync.dma_start(nrm1_t[:], nrm1_d[:])
            nrm2_t = metap.tile([P, NBLK * capT], F32)
            nc.sync.dma_start(nrm2_t[:], nrm2_d[:])

            q2T = metap.tile([P, NPC], DT, name="q2T")       # (A2 x)^T blocks
            klc = metap.tile([P, NBLK], F32, name="klc")
            nc.vector.memset(klc[:], 0.0)

            enc_shard = dramp.tile([NPC, D], DT)             # local enc_t rows
            enc_full = dramp.tile([NPAD, D], DT, addr_space="Shared")

            def gather_group(blks, table, tableB):
                nA = len(blks) * capA * P
                gA = gathp.tile([P, len(blks) * capA, D], DT, tag="gA",
                                padded_shape=[P, GROUP * capA, D])
                c0 = blks[0] * capA * 8
                nc.gpsimd.dma_gather(
                    gA[:], table, idxA_t[:, c0:c0 + len(blks) * capA * 8],
                    nA, nA, D, single_packet=False,
                    queue_num=qctr[0] % NQ)
                qctr[0] += 1
                nB = len(blks) * capB * P
                gB = gathp.tile([P, len(blks) * capB, D], DT, tag="gB",
                                padded_shape=[P, GROUP * capB, D])
                c0 = blks[0] * capB * 8
                nc.gpsimd.dma_gather(
                    gB[:], tableB, idxB_t[:, c0:c0 + len(blks) * capB * 8],
                    nB, nB, D, single_packet=False,
                    queue_num=qctr[0] % NQ)
                qctr[0] += 1
                return gA, gB

            def tile_src(gA, gB, gS, bi, t):
                if t < capA:
                    return gA[:, bi * capA + t, :]
                if t < capA + capB:
                    return gB[:, bi * capB + (t - capA), :]
                return gS[:]

            # ================= PASS A =================
            tabA = x_tab[0:SPLIT, :]
            tabB = x_tab[SPLIT:NPAD, :]
            for blks in groups:
                gA, gB = gather_group(blks, tabA, tabB)
                for bi, b in enumerate(blks):
                    gS = blkp.tile([P, D], DT, tag="gS")
                    nc.sync.dma_start(gS[:], x_self_d[b * P:(b + 1) * P, :])
                    acc12 = psp.tile([D, 2 * P], F32, space="PSUM", tag="acc")
                    for t in range(capT):
                        col = b * capT + t
                        g = tile_src(gA, gB, gS, bi, t)
                        oh12 = ohp.tile([P, 2 * P], DT, tag="oh")
                        nc.vector.tensor_scalar(
                            oh12[:, 0:P], iota_t[:], dloc_t[:, col:col + 1],
                            nrm1_t[:, col:col + 1], AluOp.is_equal, AluOp.mult)
                        nc.vector.tensor_scalar(
                            oh12[:, P:2 * P], iota_t[:], dloc_t[:, col:col + 1],
                            nrm2_t[:, col:col + 1], AluOp.is_equal, AluOp.mult)
                        nc.tensor.matmul(acc12[:], g, oh12[:],
                                         start=(t == 0), stop=(t == capT - 1))
                    # Q2 block = (A2 x)^T
                    nc.vector.tensor_copy(q2T[:, b * P:(b + 1) * P],
                                          acc12[:, P:2 * P])
                    # enc_t block: relu(W_enc^T @ (A1 x)^T + b_enc)
                    a1xT = blkp.tile([D, P], DT, tag="a1xT")
                    nc.vector.tensor_copy(a1xT[:], acc12[:, 0:P])
                    ps_enc = ps2p.tile([D, P], F32, space="PSUM", tag="psd")
                    nc.tensor.matmul(ps_enc[:], w_t["W_enc"][:], a1xT[:],
                                     start=True, stop=True)
                    enc_tT = blkp.tile([D, P], DT, tag="enc_tT")
                    nc.scalar.activation(enc_tT[:], ps_enc[:], Act.Relu,
                                         bias=b_t["b_enc"][:], scale=1.0)
                    ps_tr = ps2p.tile([P, D], DT, space="PSUM", tag="psd")
                    nc.tensor.transpose(ps_tr[:], enc_tT[:], ident_t[:])
                    enc_row = blkp.tile([P, D], DT, tag="enc_row")
                    nc.vector.tensor_copy(enc_row[:], ps_tr[:])
                    nc.sync.dma_start(enc_shard[b * P:(b + 1) * P, :],
                                      enc_row[:])

            if stage != "A":
                # ================= ALLGATHER =================
                nc.gpsimd.collective_compute(
                    "AllGather", AluOp.bypass,
                    replica_groups=[list(range(NCORES))],
                    ins=[enc_shard[:]], outs=[enc_full[:]],
                )

            if stage not in ("A", "AG"):
                # ================= PASS B =================
                tabA2 = enc_full[0:SPLIT, :]
                tabB2 = enc_full[SPLIT:NPAD, :]
                for blks in groups:
                    gA, gB = gather_group(blks, tabA2, tabB2)
                    for bi, b in enumerate(blks):
                        gS = blkp.tile([P, D], DT, tag="gS")
                        nc.sync.dma_start(gS[:],
                                          enc_shard[b * P:(b + 1) * P, :])
                        accp = psp.tile([D, P], F32, space="PSUM", tag="acc")
                        for t in range(capT):
                            col = b * capT + t
                            g = tile_src(gA, gB, gS, bi, t)
                            oh = ohp.tile([P, 2 * P], DT, tag="oh")
                            nc.vector.tensor_scalar(
                                oh[:, 0:P], iota_t[:], dloc_t[:, col:col + 1],
                                nrm2_t[:, col:col + 1], AluOp.is_equal,
                                AluOp.mult)
                            nc.tensor.matmul(accp[:], g, oh[:, 0:P],
                                             start=(t == 0),
                                             stop=(t == capT - 1))
                        p2T = blkp.tile([D, P], DT, tag="p2T")
                        nc.vector.tensor_copy(p2T[:], accp[:])

                        # encoder head
                        ps_mu = ps2p.tile([D, P], F32, space="PSUM", tag="psd")
                        nc.tensor.matmul(ps_mu[:], w_t["W_mu"][:], p2T[:],
                                         start=True, stop=True)
                        emT = blkp.tile([D, P], F32, tag="emT")
                        nc.vector.tensor_scalar(emT[:], ps_mu[:],
                                                b_t["b_mu"][:], None, AluOp.add)
                        ps_sd = ps2p.tile([D, P], F32, space="PSUM", tag="psd")
                        nc.tensor.matmul(ps_sd[:], w_t["W_std"][:], p2T[:],
                                         start=True, stop=True)
                        esT = blkp.tile([D, P], F32, tag="esT")
                        nc.scalar.activation(esT[:], ps_sd[:], Act.Sigmoid,
                                             bias=b_t["b_std"][:], scale=1.0)

                        # conf_z = eps * enc_std + enc_mean
                        epsb = blkp.tile([P, P], F32, tag="epsb")
                        nc.sync.dma_start(epsb[:], epsT_d[:, b * P:(b + 1) * P])
                        cz = blkp.tile([P, P], F32, tag="cz")
                        nc.vector.tensor_tensor(cz[:], epsb[:], esT[:],
                                                AluOp.mult)
                        nc.vector.tensor_tensor(cz[:], cz[:], emT[:], AluOp.add)
                        nc.sync.dma_start(confzT_d[:, b * P:(b + 1) * P], cz[:])

                        # prior head
                        ps_pr = ps2p.tile([D, P], F32, space="PSUM", tag="psd")
                        nc.tensor.matmul(ps_pr[:], w_t["W_prior"][:],
                                         q2T[:, b * P:(b + 1) * P],
                                         start=True, stop=True)
                        prT = blkp.tile([D, P], DT, tag="prT")
                        nc.scalar.activation(prT[:], ps_pr[:], Act.Relu,
                                             bias=b_t["b_prior"][:], scale=1.0)
                        ps_pm = ps2p.tile([D, P], F32, space="PSUM", tag="psd")
                        nc.tensor.matmul(ps_pm[:], w_t["W_pm"][:], prT[:],
                                         start=True, stop=True)
                        pmT = blkp.tile([D, P], F32, tag="pmT")
                        nc.vector.tensor_scalar(pmT[:], ps_pm[:],
                                                b_t["bpm2"][:], None, AluOp.add)
                        ps_ps = ps2p.tile([D, P], F32, space="PSUM", tag="psd")
                        nc.tensor.matmul(ps_ps[:], w_t["W_ps"][:], prT[:],
                                         start=True, stop=True)
                        psT = blkp.tile([D, P], F32, tag="psT")
                        nc.scalar.activation(psT[:], ps_ps[:], Act.Sigmoid,
                                             bias=b_t["bps2"][:], scale=1.0)

                        # kl = 2ln(ps+e) - 2ln(es+e)
                        #      + ((es+e)^2 + (em-pm)^2)/(ps+e)^2 - 1
                        t1 = blkp.tile([P, P], F32, tag="t1")
                        nc.scalar.activation(t1[:], psT[:], Act.Ln,
                                             bias=epsc_t[:], scale=1.0)
                        t2 = blkp.tile([P, P], F32, tag="t2")
                        nc.scalar.activation(t2[:], esT[:], Act.Ln,
                                             bias=epsc_t[:], scale=1.0)
                        a2 = blkp.tile([P, P], F32, tag="a2")
                        nc.scalar.activation(a2[:], esT[:], Act.Square,
                                             bias=epsc_t[:], scale=1.0)
                        p2s = blkp.tile([P, P], F32, tag="p2s")
                        nc.scalar.activation(p2s[:], psT[:], Act.Square,
                                             bias=epsc_t[:], scale=1.0)
                        rcp = blkp.tile([P, P], F32, tag="rcp")
                        nc.vector.reciprocal(rcp[:], p2s[:])
                        dmm = blkp.tile([P, P], F32, tag="dmm")
                        nc.vector.tensor_tensor(dmm[:], emT[:], pmT[:],
                                                AluOp.subtract)
                        nc.vector.tensor_tensor(dmm[:], dmm[:], dmm[:],
                                                AluOp.mult)
                        nc.vector.tensor_tensor(a2[:], a2[:], dmm[:], AluOp.add)
                        nc.vector.tensor_tensor(a2[:], a2[:], rcp[:],
                                                AluOp.mult)
                        nc.vector.tensor_tensor(t1[:], t1[:], t2[:],
                                                AluOp.subtract)
                        klt = blkp.tile([P, P], F32, tag="klt")
                        nc.vector.tensor_scalar(t1[:], t1[:], 2.0, -1.0,
                                                AluOp.mult, AluOp.add)
                        nc.vector.tensor_tensor(klt[:], t1[:], a2[:], AluOp.add)
                        ps_kl = ps2p.tile([P, 1], F32, space="PSUM", tag="psd")
                        nc.tensor.matmul(ps_kl[:], klt[:], ones_t[:],
                                         start=True, stop=True)
                        nc.vector.tensor_copy(klc[:, b:b + 1], ps_kl[:])

            nc.sync.dma_start(klcols_d[:], klc[:])

    nc.compile()
    _PROGRAM_CACHE[key] = nc
    return nc


# ----------------------------------------------------------------------------
# entry point
# ----------------------------------------------------------------------------

def kernel(edge_index, x, t, edge_score, total_len, train_len,
           W_enc, b_enc, W_mu, b_mu, W_std, b_std,
           W_prior, b_prior, W_pm, b_pm, W_ps, b_ps, time_emb):
    from concourse.bass_utils import run_bass_kernel_spmd

    prop_np = _prop_np()
    x = np.asarray(x, np.float32)
    time_emb = np.asarray(time_emb, np.float32)
    tidx = int(t)

    pp = _preprocess(np.asarray(edge_index), np.asarray(edge_score, np.float32))
    cores = _build_core_arrays(pp, prop_np)

    # permuted feature table (rank-major new order)
    x_tab = np.zeros((NPAD, D), np.float32)
    x_tab[pp["new_row"][:N]] = x
    x_tab = x_tab.astype(prop_np)

    import jax

    with jax.default_device(jax.local_devices(backend="cpu")[0]):
        eps = np.asarray(jax.random.normal(
            jax.random.key(42), (N, D), "float32"))

    iota = np.tile(np.arange(P, dtype=np.float32)[None, :], (P, 1)).astype(prop_np)
    ident = np.eye(P, dtype=np.float32).astype(prop_np)
    te = time_emb[tidx].astype(np.float32)
    bpm2 = (np.asarray(b_pm, np.float32) + te @ np.asarray(W_pm, np.float32))
    bps2 = (np.asarray(b_ps, np.float32) + te @ np.asarray(W_ps, np.float32))

    nc = _build_program(pp["capA"], pp["capB"])

    in_maps = []
    for c in range(NCORES):
        nrow = pp["new_row"][:N]
        mine = (nrow // NPC) == c
        local = nrow[mine] % NPC
        epsT = np.zeros((P, NPC), np.float32)
        epsT[:, local] = eps[mine].T
        in_maps.append({
            "x_tab": x_tab,
            "x_self": np.ascontiguousarray(x_tab[c * NPC:(c + 1) * NPC]),
            "idxA": cores[c]["idxA"], "idxB": cores[c]["idxB"],
            "dloc": cores[c]["dloc"], "nrm1": cores[c]["nrm1"],
            "nrm2": cores[c]["nrm2"],
            "iota": iota, "ident": ident, "epsT": epsT,
            "W_enc": np.asarray(W_enc, np.float32).astype(prop_np),
            "W_mu": np.asarray(W_mu, np.float32).astype(prop_np),
            "W_std": np.asarray(W_std, np.float32).astype(prop_np),
            "W_prior": np.asarray(W_prior, np.float32).astype(prop_np),
            "W_pm": np.asarray(W_pm, np.float32).astype(prop_np),
            "W_ps": np.asarray(W_ps, np.float32).astype(prop_np),
            "b_enc": np.asarray(b_enc, np.float32).reshape(D, 1),
            "b_mu": np.asarray(b_mu, np.float32).reshape(D, 1),
            "b_std": np.asarray(b_std, np.float32).reshape(D, 1),
            "b_prior": np.asarray(b_prior, np.float32).reshape(D, 1),
            "bpm2": bpm2.reshape(D, 1), "bps2": bps2.reshape(D, 1),
        })

    if os.environ.get("GCN_SIM"):
        from concourse.bass_interp import MultiCoreSim

        sim = MultiCoreSim(nc, NCORES)
        for c in range(NCORES):
            for k, v in in_maps[c].items():
                sim.cores[c].tensor(k)[:] = v
        sim.simulate(check_with_hw=False)

        class _R:
            results = [{n: np.asarray(sim.cores[c].mem_tensor(n))
                        for n in ("confzT", "klcols")} for c in range(NCORES)]
            exec_time_ns = None
            instructions_and_trace = None

        res = _R()
    else:
        res = run_bass_kernel_spmd(nc, in_maps, list(range(NCORES)),
                                   trace=bool(os.environ.get("GCN_TRACE")))
    kernel.last_exec_time_ns = res.exec_time_ns
    kernel.last_res = res
    kernel.last_trace = (res.instructions_and_trace[1]
                         if res.instructions_and_trace else None)

    # ---- host-side assembly ----
    conf_full = np.empty((NPAD, D), np.float32)
    kl_total = 0.0
    for c in range(NCORES):
        czT = res.results[c]["confzT"]            # [128, NPC]
        conf_full[c * NPC:(c + 1) * NPC] = czT.T
        klcols = res.results[c]["klcols"]         # [128, NBLK]
        valid = np.zeros(NPC, bool)
        nrow = pp["new_row"][:N]
        mine = (nrow // NPC) == c
        valid[nrow[mine] % NPC] = True
        kl_total += float(klcols.T.reshape(NPC)[valid].astype(np.float64).sum())

    conf_z = conf_full[pp["new_row"][:N]]
    kl_loss = np.float32(0.5 * kl_total / N)
    return kl_loss, conf_z


# revision 14
# speedup vs baseline: 1.0745x; 1.0745x over previous
"""Trainium2 Bass kernel for a VGAE-style 2-layer GCN encoder (fused KL).

Math refactor: GCNConv(x, W) = A_norm @ (x @ W) + b = (A_norm @ x) @ W + b, so
only two sparse propagations are needed:
  pass A: A1x = A1 @ x (edge_score weighting), Q2 = A2 @ x (ones weighting)
          -- both from ONE gather of x[src] per edge;
          enc_t = relu(A1x @ W_enc + b_enc)
  allgather enc_t across the 8 cores
  pass B: P2 = A2 @ enc_t; then small dense heads + fused KL.

Distribution: destination-sharded, 49 blocks x 128 nodes per core (padded
50176 nodes globally, rank-major new-id order).  Sparse propagation is
edge-centric: dma_gather of source rows (4 SWDGE queues round-robin,
single_packet=False) + per-128-edge-tile scaled one-hot built in one DVE
tensor_scalar (is_equal, mult) + TensorE matmul accumulated in PSUM.  Pass A
streams both weightings as one 256-wide rhs.  Self-loops use a plain DMA of
the block's own rows instead of gather slots.
"""
import math
import os

import numpy as np

N = int(os.environ.get("GCN_N", 50000))
E = int(os.environ.get("GCN_E", 800000))
D = 128
P = 128
NCORES = 8
NBLK = int(os.environ.get("GCN_NBLK", 49))   # blocks per core
NPC = NBLK * P             # nodes per core
NPAD = NCORES * NPC        # padded node count
SPLIT = (NCORES // 2) * NPC  # gather-table A/B split row (int16 range)
EPS = 1e-9

PROP = os.environ.get("GCN_PROP_DT", "bf16")
GROUP = 4 if PROP == "bf16" else 2   # blocks per dma_gather call
NQ = 4                               # SWDGE queues


def _prop_np():
    if PROP == "bf16":
        import ml_dtypes

        return np.dtype(ml_dtypes.bfloat16)
    return np.dtype(np.float32)


# ----------------------------------------------------------------------------
# host-side graph preprocessing
# ----------------------------------------------------------------------------

def _preprocess(edge_index, edge_score):
    src = np.asarray(edge_index[0], np.int64)
    dst = np.asarray(edge_index[1], np.int64)
    w1 = np.asarray(edge_score, np.float32)

    # degrees include the self loop (weight 1 in both weightings)
    loop = np.arange(N, dtype=np.int64)
    s_all = np.concatenate([src, loop])
    d_all = np.concatenate([dst, loop])
    w_all = np.concatenate([w1, np.ones(N, np.float32)])
    deg1 = np.bincount(d_all, weights=w_all.astype(np.float64), minlength=N)
    deg2 = np.bincount(d_all, minlength=N).astype(np.float64)
    dinv1 = (1.0 / np.sqrt(deg1)).astype(np.float32)
    dinv2 = (1.0 / np.sqrt(deg2)).astype(np.float32)

    # per-REAL-edge norms; self loops handled by the per-block self tile
    norm1 = dinv1[src] * w1 * dinv1[dst]
    norm2 = dinv2[src] * dinv2[dst]
    selfn1 = dinv1 * dinv1          # weight-1 self loop under both weightings
    selfn2 = dinv2 * dinv2

    indeg = np.bincount(d_all, minlength=N).astype(np.int64)

    # -- phase 1: snake-assign nodes (padded to NPAD) to cores by in-degree --
    deg_pad = np.zeros(NPAD, np.int64)
    deg_pad[:N] = indeg
    order = np.argsort(-deg_pad, kind="stable")
    snake = np.concatenate([np.arange(NCORES), np.arange(NCORES)[::-1]])
    core_of = np.empty(NPAD, np.int32)
    core_of[order] = snake[np.arange(NPAD) % (2 * NCORES)]

    # -- phase 2: per core, pack nodes into NBLK blocks balancing (dA, dB) --
    srcA = core_of[src] < (NCORES // 2)
    dA = np.bincount(dst[srcA], minlength=N)
    dB = np.bincount(dst[~srcA], minlength=N)
    dA_pad = np.zeros(NPAD, np.int64)
    dB_pad = np.zeros(NPAD, np.int64)
    dA_pad[:N] = dA
    dB_pad[:N] = dB

    block_of = np.empty(NPAD, np.int32)
    slot_of = np.empty(NPAD, np.int32)
    for c in range(NCORES):
        ids = np.nonzero(core_of == c)[0]
        assert len(ids) == NPC, len(ids)
        ids = ids[np.argsort(-(dA_pad[ids] + dB_pad[ids]), kind="stable")]
        loadA = np.zeros(NBLK, np.int64)
        loadB = np.zeros(NBLK, np.int64)
        count = np.zeros(NBLK, np.int32)
        for nid in ids:
            a, b = dA_pad[nid], dB_pad[nid]
            cand = np.nonzero(count < P)[0]
            j = cand[np.argmin(np.maximum(loadA[cand] + a, loadB[cand] + b))]
            block_of[nid] = j
            slot_of[nid] = count[j]
            count[j] += 1
            loadA[j] += a
            loadB[j] += b
    new_row = core_of.astype(np.int64) * NPC + block_of * P + slot_of

    capA = int(math.ceil(max(
        np.bincount((core_of[dst] * NBLK + block_of[dst])[srcA],
                    minlength=NCORES * NBLK).max(), 1) / P))
    capB = int(math.ceil(max(
        np.bincount((core_of[dst] * NBLK + block_of[dst])[~srcA],
                    minlength=NCORES * NBLK).max(), 1) / P))

    # -- real-edge arrays sorted by (core, block, group) --
    key = (core_of[dst].astype(np.int64) * NBLK + block_of[dst]) * 2 + (
        ~srcA).astype(np.int64)
    eorder = np.argsort(key, kind="stable")
    key_s = key[eorder]
    return dict(
        new_row=new_row, core_of=core_of, block_of=block_of, slot_of=slot_of,
        capA=capA, capB=capB,
        srow=new_row[src][eorder], dloc=slot_of[dst][eorder],
        n1=norm1[eorder], n2=norm2[eorder],
        seg_starts=np.searchsorted(key_s, np.arange(NCORES * NBLK * 2)),
        seg_ends=np.searchsorted(key_s, np.arange(NCORES * NBLK * 2) + 1),
        selfn1=selfn1, selfn2=selfn2,
    )


def _wrap_idxs(a):
    """[num] -> [128, num/16] int16 wrapped + replicated-per-Q7-pair layout."""
    return np.tile(np.asarray(a, np.int16).reshape(-1, 16).T, (8, 1))


def _build_core_arrays(pp, prop_np):
    """Per-core idx/dstloc/norm arrays; capT = capA + capB + 1 (self tile)."""
    capA, capB = pp["capA"], pp["capB"]
    capT = capA + capB + 1
    # map (core,block,slot) -> old node id (or -1 for dummy slots)
    old_of = np.full(NPAD, -1, np.int64)
    old_of[pp["new_row"][:N] if N < NPAD else pp["new_row"]] = np.arange(
        min(N, NPAD))
    out = []
    for c in range(NCORES):
        idxA = np.zeros((NBLK, capA * P), np.int16)
        idxB = np.zeros((NBLK, capB * P), np.int16)
        dloc = np.zeros((P, NBLK * capT), np.float32)
        nrm1 = np.zeros((P, NBLK * capT), np.float32)
        nrm2 = np.zeros((P, NBLK * capT), np.float32)
        for b in range(NBLK):
            for grp in range(2):
                seg = (c * NBLK + b) * 2 + grp
                s0, s1 = pp["seg_starts"][seg], pp["seg_ends"][seg]
                n = s1 - s0
                cap = capA if grp == 0 else capB
                assert n <= cap * P, (c, b, grp, n, cap * P)
                rows = pp["srow"][s0:s1] - (0 if grp == 0 else SPLIT)
                (idxA if grp == 0 else idxB)[b, :n] = rows
                colbase = b * capT + (0 if grp == 0 else capA)
                dl = np.zeros(cap * P, np.float32)
                w1 = np.zeros(cap * P, np.float32)
                w2 = np.zeros(cap * P, np.float32)
                dl[:n] = pp["dloc"][s0:s1]
                w1[:n] = pp["n1"][s0:s1]
                w2[:n] = pp["n2"][s0:s1]
                dloc[:, colbase:colbase + cap] = dl.reshape(cap, P).T
                nrm1[:, colbase:colbase + cap] = w1.reshape(cap, P).T
                nrm2[:, colbase:colbase + cap] = w2.reshape(cap, P).T
            # self tile (last col of the block): lane p == slot p
            sc = b * capT + capT - 1
            dloc[:, sc] = np.arange(P, dtype=np.float32)
            olds = old_of[c * NPC + b * P + np.arange(P)]
            valid = olds >= 0
            nrm1[valid, sc] = pp["selfn1"][olds[valid]]
            nrm2[valid, sc] = pp["selfn2"][olds[valid]]
        out.append(dict(
            idxA=_wrap_idxs(idxA.reshape(-1)),
            idxB=_wrap_idxs(idxB.reshape(-1)),
            dloc=dloc, nrm1=nrm1, nrm2=nrm2,
        ))
    return out


# ----------------------------------------------------------------------------
# bass program
# ----------------------------------------------------------------------------

_PROGRAM_CACHE = {}


def _build_program(capA, capB):
    stage = os.environ.get("GCN_STAGE", "full")
    key = (capA, capB, PROP, stage)
    if key in _PROGRAM_CACHE:
        return _PROGRAM_CACHE[key]

    import concourse.bacc as bacc
    import concourse.tile as tile
    from concourse import mybir

    DT = mybir.dt.bfloat16 if PROP == "bf16" else mybir.dt.float32
    F32 = mybir.dt.float32
    I16 = mybir.dt.int16
    capT = capA + capB + 1
    AluOp = mybir.AluOpType
    Act = mybir.ActivationFunctionType

    nc = bacc.Bacc("TRN2", target_bir_lowering=False, debug=False,
                   num_devices=NCORES, num_swdge_queues=NQ)

    # inputs
    x_tab = nc.dram_tensor("x_tab", [NPAD, D], DT, kind="ExternalInput")
    x_self_d = nc.dram_tensor("x_self", [NPC, D], DT, kind="ExternalInput")
    idxA_d = nc.dram_tensor("idxA", [128, NBLK * capA * 8], I16, kind="ExternalInput")
    idxB_d = nc.dram_tensor("idxB", [128, NBLK * capB * 8], I16, kind="ExternalInput")
    dloc_d = nc.dram_tensor("dloc", [P, NBLK * capT], F32, kind="ExternalInput")
    nrm1_d = nc.dram_tensor("nrm1", [P, NBLK * capT], F32, kind="ExternalInput")
    nrm2_d = nc.dram_tensor("nrm2", [P, NBLK * capT], F32, kind="ExternalInput")
    iota_d = nc.dram_tensor("iota", [P, P], DT, kind="ExternalInput")
    ident_d = nc.dram_tensor("ident", [P, P], DT, kind="ExternalInput")
    epsT_d = nc.dram_tensor("epsT", [P, NPC], F32, kind="ExternalInput")
    w_names = ["W_enc", "W_mu", "W_std", "W_prior", "W_pm", "W_ps"]
    w_d = {n: nc.dram_tensor(n, [D, D], DT, kind="ExternalInput") for n in w_names}
    b_names = ["b_enc", "b_mu", "b_std", "b_prior", "bpm2", "bps2"]
    b_d = {n: nc.dram_tensor(n, [D, 1], F32, kind="ExternalInput") for n in b_names}

    # outputs
    confzT_d = nc.dram_tensor("confzT", [P, NPC], F32, kind="ExternalOutput")
    klcols_d = nc.dram_tensor("klcols", [P, NBLK], F32, kind="ExternalOutput")

    NGRP = (NBLK + GROUP - 1) // GROUP
    groups = [list(range(g * GROUP, min((g + 1) * GROUP, NBLK)))
              for g in range(NGRP)]
    qctr = [0]

    with tile.TileContext(nc) as tc:
        with (
            tc.tile_pool(name="const", bufs=1) as constp,
            tc.tile_pool(name="meta", bufs=1) as metap,
            tc.tile_pool(name="gath", bufs=2) as gathp,
            tc.tile_pool(name="oh", bufs=4) as ohp,
            tc.tile_pool(name="blk", bufs=3) as blkp,
            tc.tile_pool(name="ps", bufs=2, space="PSUM") as psp,
            tc.tile_pool(name="ps2", bufs=4, space="PSUM") as ps2p,
            tc.tile_pool(name="psi", bufs=1, space="PSUM") as psip,
            tc.tile_pool(name="dram", bufs=1, space="DRAM") as dramp,
        ):
            # ---- resident constants / metadata ----
            iota_t = constp.tile([P, P], DT)
            nc.sync.dma_start(iota_t[:], iota_d[:])
            iota_ps = psip.tile([P, P], F32, space="PSUM")
            nc.vector.tensor_copy(iota_ps[:], iota_t[:])
            ident_t = constp.tile([P, P], DT)
            nc.sync.dma_start(ident_t[:], ident_d[:])
            ones_t = constp.tile([P, 1], F32)
            nc.vector.memset(ones_t[:], 1.0)
            epsc_t = constp.tile([P, 1], F32)
            nc.vector.memset(epsc_t[:], EPS)
            w_t = {}
            for n in w_names:
                w_t[n] = constp.tile([D, D], DT, name=f"w_{n}")
                nc.sync.dma_start(w_t[n][:], w_d[n][:])
            b_t = {}
            for n in b_names:
                b_t[n] = constp.tile([D, 1], F32, name=f"b_{n}")
                nc.sync.dma_start(b_t[n][:], b_d[n][:])
            idxA_t = metap.tile([128, NBLK * capA * 8], I16)
            nc.sync.dma_start(idxA_t[:], idxA_d[:])
            idxB_t = metap.tile([128, NBLK * capB * 8], I16)
            nc.sync.dma_start(idxB_t[:], idxB_d[:])
            dloc_t = metap.tile([P, NBLK * capT], F32)
            nc.sync.dma_start(dloc_t[:], dloc_d[:])
            nrm1_t = metap.tile([P, NBLK * capT], F32)
            nc.sync.dma_start(nrm1_t[:], nrm1_d[:])
            nrm2_t = metap.tile([P, NBLK * capT], F32)
            nc.sync.dma_start(nrm2_t[:], nrm2_d[:])

            q2T = metap.tile([P, NPC], DT, name="q2T")       # (A2 x)^T blocks
            klc = metap.tile([P, NBLK], F32, name="klc")
            nc.vector.memset(klc[:], 0.0)

            enc_shard = dramp.tile([NPC, D], DT)             # local enc_t rows
            enc_full = dramp.tile([NPAD, D], DT, addr_space="Shared")

            def gather_group(blks, table, tableB):
                nA = len(blks) * capA * P
                gA = gathp.tile([P, len(blks) * capA, D], DT, tag="gA",
                                padded_shape=[P, GROUP * capA, D])
                c0 = blks[0] * capA * 8
                nc.gpsimd.dma_gather(
                    gA[:], table, idxA_t[:, c0:c0 + len(blks) * capA * 8],
                    nA, nA, D, single_packet=False,
                    queue_num=qctr[0] % NQ)
                qctr[0] += 1
                nB = len(blks) * capB * P
                gB = gathp.tile([P, len(blks) * capB, D], DT, tag="gB",
                                padded_shape=[P, GROUP * capB, D])
                c0 = blks[0] * capB * 8
                nc.gpsimd.dma_gather(
                    gB[:], tableB, idxB_t[:, c0:c0 + len(blks) * capB * 8],
                    nB, nB, D, single_packet=False,
                    queue_num=qctr[0] % NQ)
                qctr[0] += 1
                return gA, gB

            def tile_src(gA, gB, gS, bi, t):
                if t < capA:
                    return gA[:, bi * capA + t, :]
                if t < capA + capB:
                    return gB[:, bi * capB + (t - capA), :]
                return gS[:]

            # ================= PASS A =================
            tabA = x_tab[0:SPLIT, :]
            tabB = x_tab[SPLIT:NPAD, :]
            for blks in groups:
                gA, gB = gather_group(blks, tabA, tabB)
                for bi, b in enumerate(blks):
                    gS = blkp.tile([P, D], DT, tag="gS")
                    nc.sync.dma_start(gS[:], x_self_d[b * P:(b + 1) * P, :])
                    acc12 = psp.tile([D, 2 * P], F32, space="PSUM", tag="acc")
                    for t in range(capT):
                        col = b * capT + t
                        g = tile_src(gA, gB, gS, bi, t)
                        oh12 = ohp.tile([P, 2 * P], DT, tag="oh")
                        nc.vector.tensor_scalar(
                            oh12[:, 0:P], iota_ps[:], dloc_t[:, col:col + 1],
                            nrm1_t[:, col:col + 1], AluOp.is_equal, AluOp.mult)
                        nc.vector.tensor_scalar(
                            oh12[:, P:2 * P], iota_ps[:], dloc_t[:, col:col + 1],
                            nrm2_t[:, col:col + 1], AluOp.is_equal, AluOp.mult)
                        nc.tensor.matmul(acc12[:], g, oh12[:],
                                         start=(t == 0), stop=(t == capT - 1))
                    # Q2 block = (A2 x)^T
                    nc.vector.tensor_copy(q2T[:, b * P:(b + 1) * P],
                                          acc12[:, P:2 * P])
                    # enc_t block: relu(W_enc^T @ (A1 x)^T + b_enc)
                    a1xT = blkp.tile([D, P], DT, tag="a1xT")
                    nc.vector.tensor_copy(a1xT[:], acc12[:, 0:P])
                    ps_enc = ps2p.tile([D, P], F32, space="PSUM", tag="psd")
                    nc.tensor.matmul(ps_enc[:], w_t["W_enc"][:], a1xT[:],
                                     start=True, stop=True)
                    enc_tT = blkp.tile([D, P], DT, tag="enc_tT")
                    nc.scalar.activation(enc_tT[:], ps_enc[:], Act.Relu,
                                         bias=b_t["b_enc"][:], scale=1.0)
                    ps_tr = ps2p.tile([P, D], DT, space="PSUM", tag="psd")
                    nc.tensor.transpose(ps_tr[:], enc_tT[:], ident_t[:])
                    enc_row = blkp.tile([P, D], DT, tag="enc_row")
                    nc.vector.tensor_copy(enc_row[:], ps_tr[:])
                    nc.sync.dma_start(enc_shard[b * P:(b + 1) * P, :],
                                      enc_row[:])

            if stage != "A":
                # ================= ALLGATHER =================
                nc.gpsimd.collective_compute(
                    "AllGather", AluOp.bypass,
                    replica_groups=[list(range(NCORES))],
                    ins=[enc_shard[:]], outs=[enc_full[:]],
                )

            if stage not in ("A", "AG"):
                # ================= PASS B =================
                tabA2 = enc_full[0:SPLIT, :]
                tabB2 = enc_full[SPLIT:NPAD, :]
                for blks in groups:
                    gA, gB = gather_group(blks, tabA2, tabB2)
                    for bi, b in enumerate(blks):
                        gS = blkp.tile([P, D], DT, tag="gS")
                        nc.sync.dma_start(gS[:],
                                          enc_shard[b * P:(b + 1) * P, :])
                        accp = psp.tile([D, P], F32, space="PSUM", tag="acc")
                        for t in range(capT):
                            col = b * capT + t
                            g = tile_src(gA, gB, gS, bi, t)
                            oh = ohp.tile([P, 2 * P], DT, tag="oh")
                            nc.vector.tensor_scalar(
                                oh[:, 0:P], iota_ps[:], dloc_t[:, col:col + 1],
                                nrm2_t[:, col:col + 1], AluOp.is_equal,
                                AluOp.mult)
                            nc.tensor.matmul(accp[:], g, oh[:, 0:P],
                                             start=(t == 0),
                                             stop=(t == capT - 1))
                        p2T = blkp.tile([D, P], DT, tag="p2T")
                        nc.vector.tensor_copy(p2T[:], accp[:])

                        # encoder head
                        ps_mu = ps2p.tile([D, P], F32, space="PSUM", tag="psd")
                        nc.tensor.matmul(ps_mu[:], w_t["W_mu"][:], p2T[:],
                                         start=True, stop=True)
                        emT = blkp.tile([D, P], F32, tag="emT")
                        nc.vector.tensor_scalar(emT[:], ps_mu[:],
                                                b_t["b_mu"][:], None, AluOp.add)
                        ps_sd = ps2p.tile([D, P], F32, space="PSUM", tag="psd")
                        nc.tensor.matmul(ps_sd[:], w_t["W_std"][:], p2T[:],
                                         start=True, stop=True)
                        esT = blkp.tile([D, P], F32, tag="esT")
                        nc.scalar.activation(esT[:], ps_sd[:], Act.Sigmoid,
                                             bias=b_t["b_std"][:], scale=1.0)

                        # conf_z = eps * enc_std + enc_mean
                        epsb = blkp.tile([P, P], F32, tag="epsb")
                        nc.sync.dma_start(epsb[:], epsT_d[:, b * P:(b + 1) * P])
                        cz = blkp.tile([P, P], F32, tag="cz")
                        nc.vector.tensor_tensor(cz[:], epsb[:], esT[:],
                                                AluOp.mult)
                        nc.vector.tensor_tensor(cz[:], cz[:], emT[:], AluOp.add)
                        nc.sync.dma_start(confzT_d[:, b * P:(b + 1) * P], cz[:])

                        # prior head
                        ps_pr = ps2p.tile([D, P], F32, space="PSUM", tag="psd")
                        nc.tensor.matmul(ps_pr[:], w_t["W_prior"][:],
                                         q2T[:, b * P:(b + 1) * P],
                                         start=True, stop=True)
                        prT = blkp.tile([D, P], DT, tag="prT")
                        nc.scalar.activation(prT[:], ps_pr[:], Act.Relu,
                                             bias=b_t["b_prior"][:], scale=1.0)
                        ps_pm = ps2p.tile([D, P], F32, space="PSUM", tag="psd")
                        nc.tensor.matmul(ps_pm[:], w_t["W_pm"][:], prT[:],
                                         start=True, stop=True)
                        pmT = blkp.tile([D, P], F32, tag="pmT")
                        nc.vector.tensor_scalar(pmT[:], ps_pm[:],
                                                b_t["bpm2"][:], None, AluOp.add)
                        ps_ps = ps2p.tile([D, P], F32, space="PSUM", tag="psd")
                        nc.tensor.matmul(ps_ps[:], w_t["W_ps"][:], prT[:],
                                         start=True, stop=True)
                        psT = blkp.tile([D, P], F32, tag="psT")
                        nc.scalar.activation(psT[:], ps_ps[:], Act.Sigmoid,
                                             bias=b_t["bps2"][:], scale=1.0)

                        # kl = 2ln(ps+e) - 2ln(es+e)
                        #      + ((es+e)^2 + (em-pm)^2)/(ps+e)^2 - 1
                        t1 = blkp.tile([P, P], F32, tag="t1")
                        nc.scalar.activation(t1[:], psT[:], Act.Ln,
                                             bias=epsc_t[:], scale=1.0)
                        t2 = blkp.tile([P, P], F32, tag="t2")
                        nc.scalar.activation(t2[:], esT[:], Act.Ln,
                                             bias=epsc_t[:], scale=1.0)
                        a2 = blkp.tile([P, P], F32, tag="a2")
                        nc.scalar.activation(a2[:], esT[:], Act.Square,
                                             bias=epsc_t[:], scale=1.0)
                        p2s = blkp.tile([P, P], F32, tag="p2s")
                        nc.scalar.activation(p2s[:], psT[:], Act.Square,
                                             bias=epsc_t[:], scale=1.0)
                        rcp = blkp.tile([P, P], F32, tag="rcp")
                        nc.vector.reciprocal(rcp[:], p2s[:])
                        dmm = blkp.tile([P, P], F32, tag="dmm")
                        nc.vector.tensor_tensor(dmm[:], emT[:], pmT[:],
                                                AluOp.subtract)
                        nc.vector.tensor_tensor(dmm[:], dmm[:], dmm[:],
                                                AluOp.mult)
                        nc.vector.tensor_tensor(a2[:], a2[:], dmm[:], AluOp.add)
                        nc.vector.tensor_tensor(a2[:], a2[:], rcp[:],
                                                AluOp.mult)
                        nc.vector.tensor_tensor(t1[:], t1[:], t2[:],
                                                AluOp.subtract)
                        klt = blkp.tile([P, P], F32, tag="klt")
                        nc.vector.tensor_scalar(t1[:], t1[:], 2.0, -1.0,
                                                AluOp.mult, AluOp.add)
                        nc.vector.tensor_tensor(klt[:], t1[:], a2[:], AluOp.add)
                        ps_kl = ps2p.tile([P, 1], F32, space="PSUM", tag="psd")
                        nc.tensor.matmul(ps_kl[:], klt[:], ones_t[:],
                                         start=True, stop=True)
                        nc.vector.tensor_copy(klc[:, b:b + 1], ps_kl[:])

            nc.sync.dma_start(klcols_d[:], klc[:])

    nc.compile()
    _PROGRAM_CACHE[key] = nc
    return nc


# ----------------------------------------------------------------------------
# entry point
# ----------------------------------------------------------------------------

def kernel(edge_index, x, t, edge_score, total_len, train_len,
           W_enc, b_enc, W_mu, b_mu, W_std, b_std,
           W_prior, b_prior, W_pm, b_pm, W_ps, b_ps, time_emb):
    from concourse.bass_utils import run_bass_kernel_spmd

    prop_np = _prop_np()
    x = np.asarray(x, np.float32)
    time_emb = np.asarray(time_emb, np.float32)
    tidx = int(t)

    pp = _preprocess(np.asarray(edge_index), np.asarray(edge_score, np.float32))
    cores = _build_core_arrays(pp, prop_np)

    # permuted feature table (rank-major new order)
    x_tab = np.zeros((NPAD, D), np.float32)
    x_tab[pp["new_row"][:N]] = x
    x_tab = x_tab.astype(prop_np)

    import jax

    with jax.default_device(jax.local_devices(backend="cpu")[0]):
        eps = np.asarray(jax.random.normal(
            jax.random.key(42), (N, D), "float32"))

    iota = np.tile(np.arange(P, dtype=np.float32)[None, :], (P, 1)).astype(prop_np)
    ident = np.eye(P, dtype=np.float32).astype(prop_np)
    te = time_emb[tidx].astype(np.float32)
    bpm2 = (np.asarray(b_pm, np.float32) + te @ np.asarray(W_pm, np.float32))
    bps2 = (np.asarray(b_ps, np.float32) + te @ np.asarray(W_ps, np.float32))

    nc = _build_program(pp["capA"], pp["capB"])

    in_maps = []
    for c in range(NCORES):
        nrow = pp["new_row"][:N]
        mine = (nrow // NPC) == c
        local = nrow[mine] % NPC
        epsT = np.zeros((P, NPC), np.float32)
        epsT[:, local] = eps[mine].T
        in_maps.append({
            "x_tab": x_tab,
            "x_self": np.ascontiguousarray(x_tab[c * NPC:(c + 1) * NPC]),
            "idxA": cores[c]["idxA"], "idxB": cores[c]["idxB"],
            "dloc": cores[c]["dloc"], "nrm1": cores[c]["nrm1"],
            "nrm2": cores[c]["nrm2"],
            "iota": iota, "ident": ident, "epsT": epsT,
            "W_enc": np.asarray(W_enc, np.float32).astype(prop_np),
            "W_mu": np.asarray(W_mu, np.float32).astype(prop_np),
            "W_std": np.asarray(W_std, np.float32).astype(prop_np),
            "W_prior": np.asarray(W_prior, np.float32).astype(prop_np),
            "W_pm": np.asarray(W_pm, np.float32).astype(prop_np),
            "W_ps": np.asarray(W_ps, np.float32).astype(prop_np),
            "b_enc": np.asarray(b_enc, np.float32).reshape(D, 1),
            "b_mu": np.asarray(b_mu, np.float32).reshape(D, 1),
            "b_std": np.asarray(b_std, np.float32).reshape(D, 1),
            "b_prior": np.asarray(b_prior, np.float32).reshape(D, 1),
            "bpm2": bpm2.reshape(D, 1), "bps2": bps2.reshape(D, 1),
        })

    if os.environ.get("GCN_SIM"):
        from concourse.bass_interp import MultiCoreSim

        sim = MultiCoreSim(nc, NCORES)
        for c in range(NCORES):
            for k, v in in_maps[c].items():
                sim.cores[c].tensor(k)[:] = v
        sim.simulate(check_with_hw=False)

        class _R:
            results = [{n: np.asarray(sim.cores[c].mem_tensor(n))
                        for n in ("confzT", "klcols")} for c in range(NCORES)]
            exec_time_ns = None
            instructions_and_trace = None

        res = _R()
    else:
        res = run_bass_kernel_spmd(nc, in_maps, list(range(NCORES)),
                                   trace=bool(os.environ.get("GCN_TRACE")))
    kernel.last_exec_time_ns = res.exec_time_ns
    kernel.last_res = res
    kernel.last_trace = (res.instructions_and_trace[1]
                         if res.instructions_and_trace else None)

    # ---- host-side assembly ----
    conf_full = np.empty((NPAD, D), np.float32)
    kl_total = 0.0
    for c in range(NCORES):
        czT = res.results[c]["confzT"]            # [128, NPC]
        conf_full[c * NPC:(c + 1) * NPC] = czT.T
        klcols = res.results[c]["klcols"]         # [128, NBLK]
        valid = np.zeros(NPC, bool)
        nrow = pp["new_row"][:N]
        mine = (nrow // NPC) == c
        valid[nrow[mine] % NPC] = True
        kl_total += float(klcols.T.reshape(NPC)[valid].astype(np.float64).sum())

    conf_z = conf_full[pp["new_row"][:N]]
    kl_loss = np.float32(0.5 * kl_total / N)
    return kl_loss, conf_z
